# revision 9
# baseline (speedup 1.0000x reference)
"""Trainium2 Bass kernel for nn_MemoryAttention (causal single-head attention
with SiLU-gated output projection), sequence-parallel across 8 NeuronCores.

Layout strategy (per core c):
  - q rows owned: 4 tiles of 256 rows: tile t = c + 8*s for slot s in 0..3
    (strided assignment balances causal attention work across cores).
  - QKV projections computed in "transposed" layout: QT/KT = [d, s] so the
    contraction dim d sits on partitions; V in natural [s, d] layout.
  - Each core computes KT/V for its own rows, AllGathers them (bf16) across
    the 8 cores, and additionally computes KT/V for the first B_DUP kv
    blocks locally to overlap compute with the collective.
  - Attention per (slot, kv-block) visit: LT = K @ QT^T accumulated in PSUM
    (kv on partitions, q on free dim), PT = exp(LT/32) * mask (mask streamed
    from a per-core host tensor; uniform instruction stream across cores),
    HT[d, q] += V^T @ PT accumulated in PSUM, row sums via ones-matmul.
  - Epilogue per slot: recip(sums) broadcast, divide, SiLU, output proj
    against wv2 producing O^T; host reassembles/transposes.
"""

import numpy as np
import ml_dtypes

import concourse.bass as bass
import concourse.tile as tile
from concourse import bacc, mybir
from concourse.bass_utils import run_bass_kernel_spmd

P = 128
D = 1024
SEQ = 8192
NCORES = 8
NSLOTS = 4          # q tiles per core, 256 rows each
QT_COLS = NSLOTS * 256
NBLK = SEQ // P     # 64 global kv blocks
B_DUP = 24          # kv blocks computed locally to overlap the collective
BLK_ELEMS = P * 8 * P  # elements in one blocked kv payload (128*8*128)

F32 = mybir.dt.float32
BF16 = mybir.dt.bfloat16
AF = mybir.ActivationFunctionType


def visit_list():
    return [(s, j) for s in range(NSLOTS) for j in range(16 * (s + 1))]


def _load_cast(nc, pool_f32, pool_bf, ext, tag):
    """DRAM [1024, N] fp32 -> SBUF [128, 8, N] bf16 (d on partitions)."""
    n = ext.shape[1]
    t32 = pool_f32.tile([P, 8, n], F32, tag="ld_f32")
    nc.sync.dma_start(out=t32, in_=ext[:].rearrange("(sub p) s -> p sub s", p=P))
    tbf = pool_bf.tile([P, 8, n], BF16, tag=tag + "_bf")
    nc.vector.tensor_copy(out=tbf, in_=t32)
    return tbf


def build_kernel(b_dup=B_DUP):
    nc = bacc.Bacc(None, target_bir_lowering=False, num_devices=NCORES)

    xq_ext = nc.declare_dram_parameter("xq", [D, QT_COLS], F32, isOutput=False)
    xd_ext = nc.declare_dram_parameter("xd", [D, b_dup * P], F32, isOutput=False)
    wq_ext = nc.declare_dram_parameter("wq", [D, D], F32, isOutput=False)
    wk_ext = nc.declare_dram_parameter("wk", [D, D], F32, isOutput=False)
    wv1_ext = nc.declare_dram_parameter("wv1", [D, D], F32, isOutput=False)
    wv2_ext = nc.declare_dram_parameter("wv2", [D, D], F32, isOutput=False)
    mask_ext = nc.declare_dram_parameter(
        "masks", [len(visit_list()), P, 256], BF16, isOutput=False
    )
    ot_ext = nc.declare_dram_parameter("ot", [D, QT_COLS], F32, isOutput=True)

    # blocked kv payloads: [kind(kt=0,v=1)][slot][half][128][8][128]
    kv_local = nc.dram_tensor("kv_local", [2, NSLOTS, 2, P, 8, P], BF16)
    kv_gath = nc.dram_tensor(
        "kv_gath", [NCORES, 2, NSLOTS, 2, P, 8, P], BF16, addr_space="Shared"
    )
    kv_dup = nc.dram_tensor("kv_dup", [b_dup, 2, P, 8, P], BF16)

    ot_view = ot_ext[:].rearrange(
        "(pr tw p) (s q) -> pr tw p s q", tw=2, p=P, q=256
    )

    with tile.TileContext(nc) as tc:
        singles_ctx = tc.tile_pool(name="singles", bufs=1)
        singles = singles_ctx.__enter__()
        with (
            tc.tile_pool(name="projpersist", bufs=1) as projpersist,
            tc.tile_pool(name="ldtmp", bufs=1) as ldtmp,
            tc.tile_pool(name="xstream", bufs=2) as xstream,
            tc.tile_pool(name="projout", bufs=3) as projout,
            tc.tile_pool(name="ppsum", bufs=4, space="PSUM") as ppsum,
        ):
            # ---- phase 0: weights + own x columns -------------------------
            wk_bf = _load_cast(nc, ldtmp, projpersist, wk_ext, "wk")
            wv1_bf = _load_cast(nc, ldtmp, projpersist, wv1_ext, "wv1")
            xq_bf = _load_cast(nc, ldtmp, projpersist, xq_ext, "xq")

            ones_sb = singles.tile([P, 1], BF16)
            nc.vector.memset(ones_sb, 1.0)
            # for zeroing a full PSUM bank via a K=1 matmul (start=True sets
            # has_written for the whole bank so later matmuls can accumulate
            # with start=False; a per-region start=True would wipe the other
            # region's has_written bits in the same bank)
            zcol_sb = singles.tile([1, P], BF16)
            nc.vector.memset(zcol_sb, 0.0)
            zrow_sb = singles.tile([1, 512], BF16)
            nc.vector.memset(zrow_sb, 0.0)

            # ---- phase 1: own KT/V -> kv_local ---------------------------
            # KT: out [dout, own_s]; lhsT=wk, rhs=xq
            for m in range(8):
                for n in range(2):
                    acc = ppsum.tile([P, 512], F32, tag="proj")
                    for sub in range(8):
                        nc.tensor.matmul(
                            acc,
                            lhsT=wk_bf[:, sub, m * P : (m + 1) * P],
                            rhs=xq_bf[:, sub, n * 512 : (n + 1) * 512],
                            start=(sub == 0),
                            stop=(sub == 7),
                        )
                    kt_out = projout.tile([P, 512], BF16, tag="kt_out")
                    nc.any.tensor_copy(out=kt_out, in_=acc)
                    for b in range(4):
                        blk = n * 4 + b
                        nc.sync.dma_start(
                            out=kv_local[0, blk // 2, blk % 2, :, m, :],
                            in_=kt_out[:, b * P : (b + 1) * P],
                        )
            # V: out [own_s, dout]; lhsT=xq chunk, rhs=wv1
            for blk in range(8):
                v_out = projout.tile([P, 1024], BF16, tag="v_out")
                for h2 in range(2):
                    acc = ppsum.tile([P, 512], F32, tag="proj")
                    for sub in range(8):
                        nc.tensor.matmul(
                            acc,
                            lhsT=xq_bf[:, sub, blk * P : (blk + 1) * P],
                            rhs=wv1_bf[:, sub, h2 * 512 : (h2 + 1) * 512],
                            start=(sub == 0),
                            stop=(sub == 7),
                        )
                    nc.any.tensor_copy(out=v_out[:, h2 * 512 : (h2 + 1) * 512], in_=acc)
                nc.sync.dma_start(
                    out=kv_local[1, blk // 2, blk % 2].rearrange("p m c -> p (m c)"),
                    in_=v_out,
                )

            # ---- phase 2: AllGather (overlaps everything below) ----------
            nc.gpsimd.collective_compute(
                "AllGather",
                mybir.AluOpType.bypass,
                replica_groups=[list(range(NCORES))],
                ins=[kv_local[:]],
                outs=[kv_gath[:]],
            )

            # ---- phase 3: QT (own q rows) --------------------------------
            wq_bf = _load_cast(nc, ldtmp, projpersist, wq_ext, "wq")
            qt_sb = singles.tile([P, 8, QT_COLS], BF16)
            for m in range(8):
                for n in range(2):
                    acc = ppsum.tile([P, 512], F32, tag="proj")
                    for sub in range(8):
                        nc.tensor.matmul(
                            acc,
                            lhsT=wq_bf[:, sub, m * P : (m + 1) * P],
                            rhs=xq_bf[:, sub, n * 512 : (n + 1) * 512],
                            start=(sub == 0),
                            stop=(sub == 7),
                        )
                    nc.any.tensor_copy(
                        out=qt_sb[:, m, n * 512 : (n + 1) * 512], in_=acc
                    )

            # ---- phase 4: duplicated KV prefix -> kv_dup -----------------
            for n in range(b_dup // 4):
                xd32 = xstream.tile([P, 8, 512], F32, tag="xd32")
                nc.sync.dma_start(
                    out=xd32,
                    in_=xd_ext[:].rearrange("(sub p) s -> p sub s", p=P)[
                        :, :, n * 512 : (n + 1) * 512
                    ],
                )
                xdbf = xstream.tile([P, 8, 512], BF16, tag="xdbf")
                nc.vector.tensor_copy(out=xdbf, in_=xd32)
                for m in range(8):
                    acc = ppsum.tile([P, 512], F32, tag="proj")
                    for sub in range(8):
                        nc.tensor.matmul(
                            acc,
                            lhsT=wk_bf[:, sub, m * P : (m + 1) * P],
                            rhs=xdbf[:, sub, :],
                            start=(sub == 0),
                            stop=(sub == 7),
                        )
                    kt_out = projout.tile([P, 512], BF16, tag="kt_out")
                    nc.any.tensor_copy(out=kt_out, in_=acc)
                    for b in range(4):
                        nc.sync.dma_start(
                            out=kv_dup[n * 4 + b, 0, :, m, :],
                            in_=kt_out[:, b * P : (b + 1) * P],
                        )
                for b in range(4):
                    v_out = projout.tile([P, 1024], BF16, tag="v_out")
                    for h2 in range(2):
                        acc = ppsum.tile([P, 512], F32, tag="proj")
                        for sub in range(8):
                            nc.tensor.matmul(
                                acc,
                                lhsT=xdbf[:, sub, b * P : (b + 1) * P],
                                rhs=wv1_bf[:, sub, h2 * 512 : (h2 + 1) * 512],
                                start=(sub == 0),
                                stop=(sub == 7),
                            )
                        nc.any.tensor_copy(
                            out=v_out[:, h2 * 512 : (h2 + 1) * 512], in_=acc
                        )
                    nc.sync.dma_start(
                        out=kv_dup[n * 4 + b, 1].rearrange("p m c -> p (m c)"),
                        in_=v_out,
                    )

        # ---- phase 5: attention --------------------------------------
        wv2_loaded = False
        with (
            tc.tile_pool(name="asingles", bufs=1) as asingles,
            tc.tile_pool(name="vpool", bufs=3) as vpool,
            tc.tile_pool(name="epool", bufs=2) as epool,
            tc.tile_pool(name="gpool", bufs=2) as gpool,
            tc.tile_pool(name="ltpsum", bufs=2, space="PSUM") as ltpsum,
            tc.tile_pool(name="htpsum", bufs=1, space="PSUM") as htpsum,
            tc.tile_pool(name="spsum", bufs=1, space="PSUM") as spsum,
            tc.tile_pool(name="opsum", bufs=1, space="PSUM") as opsum,
            tc.tile_pool(name="ldtmp2", bufs=1) as ldtmp2,
        ):
            wv2_bf = _load_cast(nc, ldtmp2, asingles, wv2_ext, "wv2")

            vis_idx = 0
            for s in range(NSLOTS):
                jmax = 16 * (s + 1) - 1
                ht = [
                    htpsum.tile([P, 2, 256], F32, tag=f"ht{pair}", name=f"ht{pair}_{s}")
                    for pair in range(4)
                ]
                for pair in range(4):
                    nc.tensor.matmul(
                        ht[pair].rearrange("p a b -> p (a b)"),
                        lhsT=zcol_sb,
                        rhs=zrow_sb,
                        start=True,
                        stop=False,
                        skip_group_check=True,
                    )
                sums = spsum.tile([1, 256], F32, tag="sums")
                for j in range(16 * (s + 1)):
                    if j < b_dup:
                        kt_src = kv_dup[j, 0]
                        v_src = kv_dup[j, 1].rearrange("p m c -> p (m c)")
                    else:
                        t = j // 2
                        kt_src = kv_gath[t % 8, 0, t // 8, j % 2]
                        v_src = kv_gath[t % 8, 1, t // 8, j % 2].rearrange(
                            "p m c -> p (m c)"
                        )
                    kt_t = vpool.tile([P, 8, P], BF16, tag="kt")
                    nc.sync.dma_start(out=kt_t, in_=kt_src)
                    v_t = vpool.tile([P, 1024], BF16, tag="v")
                    nc.sync.dma_start(out=v_t, in_=v_src)
                    m_t = vpool.tile([P, 256], BF16, tag="m")
                    nc.sync.dma_start(out=m_t, in_=mask_ext[vis_idx])

                    lt = ltpsum.tile([P, 256], F32, tag="lt")
                    for sub in range(8):
                        nc.tensor.matmul(
                            lt,
                            lhsT=kt_t[:, sub, :],
                            rhs=qt_sb[:, sub, s * 256 : (s + 1) * 256],
                            start=(sub == 0),
                            stop=(sub == 7),
                        )
                    pt = vpool.tile([P, 256], BF16, tag="pt")
                    nc.scalar.activation(out=pt, in_=lt, func=AF.Exp, scale=0.03125)
                    nc.vector.tensor_mul(out=pt, in0=pt, in1=m_t)

                    for pair in range(4):
                        for tw in range(2):
                            m = 2 * pair + tw
                            nc.tensor.matmul(
                                ht[pair][:, tw, :],
                                lhsT=v_t[:, m * P : (m + 1) * P],
                                rhs=pt,
                                start=False,
                                stop=(j == jmax),
                                skip_group_check=True,
                            )
                    nc.tensor.matmul(
                        sums,
                        lhsT=ones_sb,
                        rhs=pt,
                        start=(j == 0),
                        stop=(j == jmax),
                    )
                    vis_idx += 1

                # ---- slot epilogue -----------------------------------
                recip = epool.tile([1, 256], F32, tag="recip")
                nc.vector.reciprocal(out=recip, in_=sums)
                bc = epool.tile([P, 256], F32, tag="bc")
                nc.gpsimd.partition_broadcast(out_ap=bc, in_ap=recip)
                gt = []
                for pair in range(4):
                    tmp = epool.tile([P, 2, 256], F32, tag="httmp")
                    nc.vector.tensor_tensor(
                        tmp,
                        ht[pair],
                        bc[:, None, :].to_broadcast([P, 2, 256]),
                        mybir.AluOpType.mult,
                    )
                    g = gpool.tile([P, 2, 256], BF16, tag=f"gt{pair}")
                    nc.scalar.activation(out=g, in_=tmp, func=AF.Silu)
                    gt.append(g)
                for opair in range(4):
                    po = opsum.tile([P, 2, 256], F32, tag="po")
                    for otw in range(2):
                        oc = 2 * opair + otw
                        for m in range(8):
                            nc.tensor.matmul(
                                po[:, otw, :],
                                lhsT=wv2_bf[:, m, oc * P : (oc + 1) * P],
                                rhs=gt[m // 2][:, m % 2, :],
                                start=(m == 0),
                                stop=(m == 7),
                            )
                    oo = epool.tile([P, 2, 256], F32, tag="oo")
                    nc.any.tensor_copy(out=oo, in_=po)
                    for otw in range(2):
                        nc.sync.dma_start(
                            out=ot_view[opair, otw, :, s, :], in_=oo[:, otw, :]
                        )
        singles_ctx.__exit__(None, None, None)

    nc.finalize()
    return nc


_NC_CACHE = {}


def get_nc(b_dup=B_DUP):
    if b_dup not in _NC_CACHE:
        _NC_CACHE[b_dup] = build_kernel(b_dup)
    return _NC_CACHE[b_dup]


def build_masks():
    """Canonical [128, 256] masks; per-core selection by k = 2c + 16s - j."""
    p = np.arange(P)[:, None]
    u = np.arange(256)[None, :]
    m_ones = np.ones((P, 256), np.float32)
    m0 = (p <= u).astype(np.float32)
    m1 = (p <= u - P).astype(np.float32)
    m_zero = np.zeros((P, 256), np.float32)
    canon = np.stack([m_zero, m1, m0, m_ones]).astype(ml_dtypes.bfloat16)

    vis = visit_list()
    out = []
    for c in range(NCORES):
        sel = np.array(
            [min(max(2 * c + 16 * s - j, -2), 1) + 2 for (s, j) in vis], np.int64
        )
        out.append(canon[sel])
    return out  # list of [160, 128, 256] bf16


def kernel(x, wq, wk, wv1, wv2):
    x = np.asarray(x, np.float32)
    xT = np.ascontiguousarray(x.T)  # [D, SEQ]
    masks = build_masks()
    xd = np.ascontiguousarray(xT[:, : B_DUP * P])

    in_maps = []
    for c in range(NCORES):
        xq_c = np.concatenate(
            [xT[:, 256 * (c + 8 * s) : 256 * (c + 8 * s) + 256] for s in range(NSLOTS)],
            axis=1,
        )
        in_maps.append(
            {
                "xq": np.ascontiguousarray(xq_c),
                "xd": xd,
                "wq": np.asarray(wq, np.float32),
                "wk": np.asarray(wk, np.float32),
                "wv1": np.asarray(wv1, np.float32),
                "wv2": np.asarray(wv2, np.float32),
                "masks": masks[c],
            }
        )

    nc = get_nc()
    res = run_bass_kernel_spmd(nc, in_maps, list(range(NCORES)))

    out = np.empty((SEQ, D), np.float32)
    for c in range(NCORES):
        ot = res.results[c]["ot"]  # [D, 4*256]
        for s in range(NSLOTS):
            r0 = 256 * (c + 8 * s)
            out[r0 : r0 + 256, :] = ot[:, s * 256 : (s + 1) * 256].T
    return out


# revision 11
# speedup vs baseline: 1.3381x; 1.3381x over previous
"""Trainium2 Bass kernel for nn_MemoryAttention (causal single-head attention
with SiLU-gated output projection), sequence-parallel across 8 NeuronCores.

Strategy (per core c):
  - q rows owned: 4 slots of 256 rows: tile t = c + 8*s (strided assignment
    balances causal work; every core runs an identical instruction stream).
  - QT/KT computed in [d, s] layout (contraction dim on partitions), V in
    natural [s, d]. Each core projects KT/V for its own rows, AllGathers
    them in bf16, and locally duplicates the first B_DUP kv blocks to hide
    the collective's latency.
  - Per (slot, kv-block) visit: LT[kv, q] = K @ QT accumulated in PSUM
    (lhsT = KT subtiles), PT = exp(LT/32) (* mask for the last 16 visits of
    each slot; mask tensors streamed per-core keep the instruction stream
    uniform), then PT q-chunks become the stationary operand for both
    H[q, d] += P @ V (N=512) and rowsums += P @ 1 (N=1, shares the LDW).
  - Slot epilogue: H / sums (per-partition scalar), SiLU, PE-transpose of G,
    output projection with G^T chunks stationary -> O[q, d] written directly.
"""

import numpy as np
import ml_dtypes

import concourse.bass as bass
import concourse.tile as tile
from concourse import bacc, mybir
from concourse.bass_utils import run_bass_kernel_spmd
from concourse.masks import make_identity

P = 128
D = 1024
SEQ = 8192
NCORES = 8
NSLOTS = 4
QT_COLS = NSLOTS * 256
B_DUP = 12
N_MASKED = NSLOTS * 16  # visits with j >= 16*s need a mask on some core

F32 = mybir.dt.float32
BF16 = mybir.dt.bfloat16
AF = mybir.ActivationFunctionType


def build_kernel(b_dup=B_DUP):
    nc = bacc.Bacc(None, target_bir_lowering=False, num_devices=NCORES)

    xq_ext = nc.declare_dram_parameter("xq", [D, QT_COLS], BF16, isOutput=False)
    xd_ext = nc.declare_dram_parameter("xd", [D, b_dup * P], BF16, isOutput=False)
    wq_ext = nc.declare_dram_parameter("wq", [D, D], BF16, isOutput=False)
    wk_ext = nc.declare_dram_parameter("wk", [D, D], BF16, isOutput=False)
    wv1_ext = nc.declare_dram_parameter("wv1", [D, D], BF16, isOutput=False)
    wv2_ext = nc.declare_dram_parameter("wv2", [D, D], BF16, isOutput=False)
    mask_ext = nc.declare_dram_parameter("masks", [N_MASKED, P, 256], BF16, isOutput=False)
    o_ext = nc.declare_dram_parameter("o", [NSLOTS, 2, P, D], F32, isOutput=True)

    # blocked kv payloads: [kind(kt=0,v=1)][slot][half][128][8][128]
    kv_local = nc.dram_tensor("kv_local", [2, NSLOTS, 2, P, 8, P], BF16)
    kv_gath = nc.dram_tensor(
        "kv_gath", [NCORES, 2, NSLOTS, 2, P, 8, P], BF16, addr_space="Shared"
    )
    kv_dup = nc.dram_tensor("kv_dup", [b_dup, 2, P, 8, P], BF16)

    def wload(nc, pool, ext, tag):
        t = pool.tile([P, 8, D], BF16, tag=tag, name=tag)
        nc.sync.dma_start(out=t, in_=ext[:].rearrange("(sub p) s -> p sub s", p=P))
        return t

    with tile.TileContext(nc) as tc:
        singles_ctx = tc.tile_pool(name="singles", bufs=1)
        singles = singles_ctx.__enter__()

        with (
            tc.tile_pool(name="projw", bufs=1) as projw,
            tc.tile_pool(name="xstream", bufs=2) as xstream,
            tc.tile_pool(name="projout", bufs=4) as projout,
            tc.tile_pool(name="ppsum", bufs=4, space="PSUM") as ppsum,
        ):
            wk_bf = wload(nc, projw, wk_ext, "wk")
            wv1_bf = wload(nc, projw, wv1_ext, "wv1")
            xq_bf = singles.tile([P, 8, QT_COLS], BF16)
            nc.sync.dma_start(
                out=xq_bf, in_=xq_ext[:].rearrange("(sub p) s -> p sub s", p=P)
            )

            ones_sb = singles.tile([P, 1], BF16)
            nc.vector.memset(ones_sb, 1.0)
            zcol_sb = singles.tile([1, P], BF16)
            nc.vector.memset(zcol_sb, 0.0)
            zrow_sb = singles.tile([1, 512], BF16)
            nc.vector.memset(zrow_sb, 0.0)
            ident_sb = singles.tile([P, P], BF16)
            make_identity(nc, ident_sb)

            def kt_proj(dst, w_bf, src_bf, ncols):
                # dst[blk, 0] <- KT payload: [p(dout), m, c]
                for m in range(8):
                    for n in range(ncols // 512):
                        acc = ppsum.tile([P, 512], F32, tag="proj", name="ktp")
                        for sub in range(8):
                            nc.tensor.matmul(
                                acc,
                                lhsT=w_bf[:, sub, m * P : (m + 1) * P],
                                rhs=src_bf[:, sub, n * 512 : (n + 1) * 512],
                                start=(sub == 0),
                                stop=(sub == 7),
                            )
                        kt_out = projout.tile([P, 512], BF16, tag="kt_out", name="kto")
                        nc.vector.tensor_copy(out=kt_out, in_=acc)
                        for b in range(4):
                            dst_ap = dst(n * 4 + b)
                            nc.sync.dma_start(
                                out=dst_ap[0, :, m, :],
                                in_=kt_out[:, b * P : (b + 1) * P],
                            )

            def v_proj(dst, wv_bf, src_bf, ncols):
                for blk in range(ncols // P):
                    v_out = projout.tile([P, 1024], BF16, tag="v_out", name="vo")
                    for h2 in range(2):
                        acc = ppsum.tile([P, 512], F32, tag="proj", name="vp")
                        for sub in range(8):
                            nc.tensor.matmul(
                                acc,
                                lhsT=src_bf[:, sub, blk * P : (blk + 1) * P],
                                rhs=wv_bf[:, sub, h2 * 512 : (h2 + 1) * 512],
                                start=(sub == 0),
                                stop=(sub == 7),
                            )
                        nc.vector.tensor_copy(
                            out=v_out[:, h2 * 512 : (h2 + 1) * 512], in_=acc
                        )
                    nc.sync.dma_start(
                        out=dst(blk)[1].rearrange("p m c -> p (m c)"), in_=v_out
                    )

            # ---- own KT/V -> kv_local, then AllGather ---------------------
            own_dst = lambda blk: kv_local[:, blk // 2, blk % 2]
            kt_proj(own_dst, wk_bf, xq_bf, QT_COLS)
            v_proj(own_dst, wv1_bf, xq_bf, QT_COLS)

            nc.gpsimd.collective_compute(
                "AllGather",
                mybir.AluOpType.bypass,
                replica_groups=[list(range(NCORES))],
                ins=[kv_local[:]],
                outs=[kv_gath[:]],
            )

            # ---- QT -------------------------------------------------------
            wq_bf = wload(nc, projw, wq_ext, "wq")
            qt_sb = singles.tile([P, 8, QT_COLS], BF16)
            for m in range(8):
                for n in range(2):
                    acc = ppsum.tile([P, 512], F32, tag="proj", name="qp")
                    for sub in range(8):
                        nc.tensor.matmul(
                            acc,
                            lhsT=wq_bf[:, sub, m * P : (m + 1) * P],
                            rhs=xq_bf[:, sub, n * 512 : (n + 1) * 512],
                            start=(sub == 0),
                            stop=(sub == 7),
                        )
                    nc.vector.tensor_copy(
                        out=qt_sb[:, m, n * 512 : (n + 1) * 512], in_=acc
                    )

            # ---- duplicated kv prefix ------------------------------------
            if b_dup:
                xd_bf = xstream.tile([P, 8, b_dup * P], BF16, tag="xd", name="xd")
                nc.sync.dma_start(
                    out=xd_bf, in_=xd_ext[:].rearrange("(sub p) s -> p sub s", p=P)
                )
                dup_dst = lambda blk: kv_dup[blk]
                kt_proj(dup_dst, wk_bf, xd_bf, b_dup * P)
                v_proj(dup_dst, wv1_bf, xd_bf, b_dup * P)

        # ---- attention ----------------------------------------------------
        with (
            tc.tile_pool(name="asingles", bufs=1) as asingles,
            tc.tile_pool(name="vpool", bufs=4) as vpool,
            tc.tile_pool(name="mpool", bufs=3) as mpool,
            tc.tile_pool(name="epool", bufs=2) as epool,
            tc.tile_pool(name="gpool", bufs=2) as gpool,
            tc.tile_pool(name="ltpsum", bufs=2, space="PSUM") as ltpsum,
            tc.tile_pool(name="hpsum", bufs=1, space="PSUM") as hpsum,
            tc.tile_pool(name="spsum", bufs=1, space="PSUM") as spsum,
        ):
            wv2_bf = wload(nc, asingles, wv2_ext, "wv2")

            def visit_srcs(s, j):
                if j < b_dup:
                    base = kv_dup[j]
                else:
                    t = j // 2
                    base = kv_gath[t % 8, :, t // 8, j % 2]
                return base[0], base[1].rearrange("p m c -> p (m c)")

            def load_visit(s, j):
                kt_src, v_src = visit_srcs(s, j)
                kt_t = vpool.tile([P, 8, P], BF16, tag="kt", name="kt_t")
                nc.sync.dma_start(out=kt_t, in_=kt_src)
                v_t = vpool.tile([P, 1024], BF16, tag="v", name="v_t")
                nc.sync.dma_start(out=v_t, in_=v_src)
                return kt_t, v_t

            def logits(s, j, kt_t):
                lt = ltpsum.tile([P, 256], F32, tag="lt", name="lt")
                for sub in range(8):
                    nc.tensor.matmul(
                        lt,
                        lhsT=kt_t[:, sub, :],
                        rhs=qt_sb[:, sub, s * 256 : (s + 1) * 256],
                        start=(sub == 0),
                        stop=(sub == 7),
                    )
                return lt

            def pv(s, j, lt, v_t, h, sums, jmax):
                pt = vpool.tile([P, 256], BF16, tag="pt", name="pt")
                nc.scalar.activation(out=pt, in_=lt, func=AF.Exp, scale=0.03125)
                if j >= 16 * s:
                    m_t = mpool.tile([P, 256], BF16, tag="m", name="m_t")
                    nc.sync.dma_start(out=m_t, in_=mask_ext[16 * s + (j - 16 * s)])
                    nc.vector.tensor_mul(out=pt, in0=pt, in1=m_t)
                for qc in range(2):
                    lhsT = pt[:, qc * P : (qc + 1) * P]
                    for dh in range(2):
                        nc.tensor.matmul(
                            h[qc][:, dh, :],
                            lhsT=lhsT,
                            rhs=v_t[:, dh * 512 : (dh + 1) * 512],
                            start=(j == 0),
                            stop=(j == jmax),
                        )
                    nc.tensor.matmul(
                        sums[:, qc : qc + 1],
                        lhsT=lhsT,
                        rhs=ones_sb,
                        start=False,
                        stop=(j == jmax),
                        skip_group_check=True,
                    )

            for s in range(NSLOTS):
                nv = 16 * (s + 1)
                jmax = nv - 1
                h = [
                    hpsum.tile([P, 2, 512], F32, tag=f"hq{qc}", name=f"h{qc}_{s}")
                    for qc in range(2)
                ]
                sums = spsum.tile([P, 2], F32, tag="sums", name="sums")
                nc.tensor.matmul(
                    sums,
                    lhsT=zcol_sb,
                    rhs=zrow_sb[:, :2],
                    start=True,
                    stop=False,
                    skip_group_check=True,
                )
                # software pipeline: logits of j+1 are emitted before pv of j
                kt_t, v_t = load_visit(s, 0)
                lt_prev = logits(s, 0, kt_t)
                v_prev = v_t
                for j in range(1, nv):
                    kt_t, v_t = load_visit(s, j)
                    lt = logits(s, j, kt_t)
                    pv(s, j - 1, lt_prev, v_prev, h, sums, jmax)
                    lt_prev, v_prev = lt, v_t
                pv(s, jmax, lt_prev, v_prev, h, sums, jmax)

                # ---- epilogue ----------------------------------------
                g_bf = []
                for qc in range(2):
                    recip = epool.tile([P, 1], F32, tag="recip", name="recip")
                    nc.vector.reciprocal(out=recip, in_=sums[:, qc : qc + 1])
                    g32 = epool.tile([P, 2, 512], F32, tag="g32", name="g32")
                    nc.vector.tensor_scalar_mul(
                        out=g32, in0=h[qc], scalar1=recip
                    )
                    g = gpool.tile([P, 1024], BF16, tag=f"g{qc}", name=f"g{qc}")
                    nc.scalar.activation(
                        out=g, in_=g32.rearrange("p a b -> p (a b)"), func=AF.Silu
                    )
                    g_bf.append(g)
                # transpose G -> gt [d-part, m, 256]
                gt_sb = epool.tile([P, 8, 256], BF16, tag="gt", name="gt")
                for m in range(8):
                    for qc in range(2):
                        tp = ltpsum.tile([P, 256], BF16, tag="lt", name="tp")
                        nc.tensor.transpose(
                            tp[:, :P],
                            g_bf[qc][:, m * P : (m + 1) * P],
                            ident_sb,
                        )
                        nc.vector.tensor_copy(
                            out=gt_sb[:, m, qc * P : (qc + 1) * P], in_=tp[:, :P]
                        )
                # output projection: O[q, d] via lhsT = gt chunks
                for qc in range(2):
                    op = hpsum.tile(
                        [P, 2, 512], F32, tag=f"hq{qc}", name=f"o{qc}_{s}"
                    )
                    for m in range(8):
                        for dh in range(2):
                            nc.tensor.matmul(
                                op[:, dh, :],
                                lhsT=gt_sb[:, m, qc * P : (qc + 1) * P],
                                rhs=wv2_bf[:, m, dh * 512 : (dh + 1) * 512],
                                start=(m == 0),
                                stop=(m == 7),
                            )
                    oo = epool.tile([P, 2, 512], F32, tag="oo", name="oo")
                    nc.vector.tensor_copy(out=oo, in_=op)
                    nc.sync.dma_start(
                        out=o_ext[s, qc], in_=oo.rearrange("p a b -> p (a b)")
                    )

        singles_ctx.__exit__(None, None, None)

    nc.finalize()
    return nc


_NC_CACHE = {}


def get_nc(b_dup=B_DUP):
    if b_dup not in _NC_CACHE:
        _NC_CACHE[b_dup] = build_kernel(b_dup)
    return _NC_CACHE[b_dup]


def build_masks():
    """Masks for the last 16 visits of each slot, selected per core by
    k = 2c + 16s - j: k>=1 all-visible, k==0 upper-left triangle, k==-1
    shifted triangle, k<=-2 fully masked (padded visit)."""
    p = np.arange(P)[:, None]
    u = np.arange(256)[None, :]
    m_ones = np.ones((P, 256), np.float32)
    m0 = (p <= u).astype(np.float32)
    m1 = (p <= u - P).astype(np.float32)
    m_zero = np.zeros((P, 256), np.float32)
    canon = np.stack([m_zero, m1, m0, m_ones]).astype(ml_dtypes.bfloat16)

    out = []
    for c in range(NCORES):
        sel = []
        for s in range(NSLOTS):
            for j in range(16 * s, 16 * (s + 1)):
                k = 2 * c + 16 * s - j
                sel.append(min(max(k, -2), 1) + 2)
        out.append(canon[np.array(sel, np.int64)])
    return out  # list of [64, 128, 256] bf16


def build_in_maps(x, wq, wk, wv1, wv2, b_dup=B_DUP):
    bf = ml_dtypes.bfloat16
    xT = np.ascontiguousarray(np.asarray(x, np.float32).T).astype(bf)
    masks = build_masks()
    xd = np.ascontiguousarray(xT[:, : b_dup * P])
    w = {
        "wq": np.asarray(wq, np.float32).astype(bf),
        "wk": np.asarray(wk, np.float32).astype(bf),
        "wv1": np.asarray(wv1, np.float32).astype(bf),
        "wv2": np.asarray(wv2, np.float32).astype(bf),
    }
    in_maps = []
    for c in range(NCORES):
        xq_c = np.concatenate(
            [xT[:, 256 * (c + 8 * s) : 256 * (c + 8 * s) + 256] for s in range(NSLOTS)],
            axis=1,
        )
        in_maps.append(
            {"xq": np.ascontiguousarray(xq_c), "xd": xd, "masks": masks[c], **w}
        )
    return in_maps


def assemble_out(results):
    out = np.empty((SEQ, D), np.float32)
    for c in range(NCORES):
        o = results[c]["o"]  # [4, 2, 128, 1024]
        for s in range(NSLOTS):
            r0 = 256 * (c + 8 * s)
            out[r0 : r0 + P, :] = o[s, 0]
            out[r0 + P : r0 + 256, :] = o[s, 1]
    return out


def kernel(x, wq, wk, wv1, wv2):
    in_maps = build_in_maps(x, wq, wk, wv1, wv2)
    nc = get_nc()
    res = run_bass_kernel_spmd(nc, in_maps, list(range(NCORES)))
    return assemble_out(res.results)


# revision 17
# speedup vs baseline: 1.4443x; 1.0793x over previous
"""Trainium2 Bass kernel for nn_MemoryAttention (causal single-head attention
with SiLU-gated output projection), sequence-parallel across 8 NeuronCores.

Strategy (per core c):
  - q rows owned: 4 slots of 256 rows: tile t = c + 8*s (strided assignment
    balances causal work; every core runs an identical instruction stream).
  - QT/KT computed in [d, s] layout (contraction dim on partitions), V in
    natural [s, d]. Each core projects KT/V for its own rows, AllGathers
    them in bf16, and locally duplicates the first B_DUP kv blocks to hide
    the collective's latency.
  - Per (slot, kv-block) visit: LT[kv, q] = K @ QT accumulated in PSUM
    (lhsT = KT subtiles), PT = exp(LT/32) (* mask for the last 16 visits of
    each slot; mask tensors streamed per-core keep the instruction stream
    uniform), then PT q-chunks become the stationary operand for both
    H[q, d] += P @ V (N=512) and rowsums += P @ 1 (N=1, shares the LDW).
  - Slot epilogue: H / sums (per-partition scalar), SiLU, PE-transpose of G,
    output projection with G^T chunks stationary -> O[q, d] written directly.
"""

import numpy as np
import ml_dtypes

import concourse.bass as bass
import concourse.tile as tile
from concourse import bacc, mybir
from concourse.bass_utils import run_bass_kernel_spmd
from concourse.masks import make_identity

P = 128
D = 1024
SEQ = 8192
NCORES = 8
NSLOTS = 4
QT_COLS = NSLOTS * 256
B_DUP = 8
N_MASKED = NSLOTS * 16  # visits with j >= 16*s need a mask on some core

F32 = mybir.dt.float32
BF16 = mybir.dt.bfloat16
AF = mybir.ActivationFunctionType


def build_kernel(b_dup=B_DUP):
    nc = bacc.Bacc(None, target_bir_lowering=False, num_devices=NCORES)

    xq_ext = nc.declare_dram_parameter("xq", [D, QT_COLS], BF16, isOutput=False)
    xd_ext = nc.declare_dram_parameter("xd", [D, b_dup * P], BF16, isOutput=False)
    wq_ext = nc.declare_dram_parameter("wq", [D, D], BF16, isOutput=False)
    wk_ext = nc.declare_dram_parameter("wk", [D, D], BF16, isOutput=False)
    wv1_ext = nc.declare_dram_parameter("wv1", [D, D], BF16, isOutput=False)
    wv2_ext = nc.declare_dram_parameter("wv2", [D, D], BF16, isOutput=False)
    mask_ext = nc.declare_dram_parameter("masks", [N_MASKED, P, 256], BF16, isOutput=False)
    o_ext = nc.declare_dram_parameter("o", [NSLOTS, 2, P, D], F32, isOutput=True)

    # blocked kv payloads: [grp][kind(kt=0,v=1)][slot-in-grp][half][128][8][128]
    # split into two slot-group collectives so blocks 0-31 arrive early
    kv_local = nc.dram_tensor("kv_local", [2, 2, 2, 2, P, 8, P], BF16)
    kv_gathA = nc.dram_tensor(
        "kv_gathA", [NCORES, 2, 2, 2, P, 8, P], BF16, addr_space="Shared"
    )
    kv_gathB = nc.dram_tensor(
        "kv_gathB", [NCORES, 2, 2, 2, P, 8, P], BF16, addr_space="Shared"
    )
    kv_dup = nc.dram_tensor("kv_dup", [b_dup, 2, P, 8, P], BF16)

    def wload(nc, pool, ext, tag):
        t = pool.tile([P, 8, D], BF16, tag=tag, name=tag)
        nc.sync.dma_start(out=t, in_=ext[:].rearrange("(sub p) s -> p sub s", p=P))
        return t

    with tile.TileContext(nc) as tc:
        singles_ctx = tc.tile_pool(name="singles", bufs=1)
        singles = singles_ctx.__enter__()

        with (
            tc.tile_pool(name="projw", bufs=1) as projw,
            tc.tile_pool(name="xstream", bufs=2) as xstream,
            tc.tile_pool(name="projout", bufs=4) as projout,
            tc.tile_pool(name="ppsum", bufs=4, space="PSUM") as ppsum,
        ):
            wk_bf = wload(nc, projw, wk_ext, "wk")
            wv1_bf = wload(nc, projw, wv1_ext, "wv1")
            xq_bf = singles.tile([P, 8, QT_COLS], BF16)
            nc.sync.dma_start(
                out=xq_bf, in_=xq_ext[:].rearrange("(sub p) s -> p sub s", p=P)
            )

            ones_sb = singles.tile([P, 1], BF16)
            nc.vector.memset(ones_sb, 1.0)
            zcol_sb = singles.tile([1, P], BF16)
            nc.vector.memset(zcol_sb, 0.0)
            zrow_sb = singles.tile([1, 512], BF16)
            nc.vector.memset(zrow_sb, 0.0)
            ident_sb = singles.tile([P, P], BF16)
            make_identity(nc, ident_sb)

            def kt_proj(dst, w_bf, src_bf, col0, col1):
                # dst(blk)[0] <- KT payload: [p(dout), m, c]
                for m in range(8):
                    for n in range(col0 // 512, col1 // 512):
                        acc = ppsum.tile([P, 512], F32, tag="proj", name="ktp")
                        for sub in range(8):
                            nc.tensor.matmul(
                                acc,
                                lhsT=w_bf[:, sub, m * P : (m + 1) * P],
                                rhs=src_bf[:, sub, n * 512 : (n + 1) * 512],
                                start=(sub == 0),
                                stop=(sub == 7),
                            )
                        kt_out = projout.tile([P, 512], BF16, tag="kt_out", name="kto")
                        nc.vector.tensor_copy(out=kt_out, in_=acc)
                        for b in range(4):
                            dst_ap = dst(n * 4 + b)
                            nc.sync.dma_start(
                                out=dst_ap[0, :, m, :],
                                in_=kt_out[:, b * P : (b + 1) * P],
                            )

            def v_proj(dst, wv_bf, src_bf, col0, col1):
                for blk in range(col0 // P, col1 // P):
                    v_out = projout.tile([P, 1024], BF16, tag="v_out", name="vo")
                    for h2 in range(2):
                        acc = ppsum.tile([P, 512], F32, tag="proj", name="vp")
                        for sub in range(8):
                            nc.tensor.matmul(
                                acc,
                                lhsT=src_bf[:, sub, blk * P : (blk + 1) * P],
                                rhs=wv_bf[:, sub, h2 * 512 : (h2 + 1) * 512],
                                start=(sub == 0),
                                stop=(sub == 7),
                            )
                        nc.vector.tensor_copy(
                            out=v_out[:, h2 * 512 : (h2 + 1) * 512], in_=acc
                        )
                    nc.sync.dma_start(
                        out=dst(blk)[1].rearrange("p m c -> p (m c)"), in_=v_out
                    )

            # ---- own KT/V -> kv_local; gather each slot-group asap -------
            own_dst = lambda blk: kv_local[blk // 4, :, (blk // 2) % 2, blk % 2]
            for grp, gath in ((0, kv_gathA), (1, kv_gathB)):
                kt_proj(own_dst, wk_bf, xq_bf, grp * 512, (grp + 1) * 512)
                v_proj(own_dst, wv1_bf, xq_bf, grp * 512, (grp + 1) * 512)
                nc.gpsimd.collective_compute(
                    "AllGather",
                    mybir.AluOpType.bypass,
                    replica_groups=[list(range(NCORES))],
                    ins=[kv_local[grp]],
                    outs=[gath[:]],
                )

            # ---- QT -------------------------------------------------------
            wq_bf = wload(nc, projw, wq_ext, "wq")
            qt_sb = singles.tile([P, 8, QT_COLS], BF16)
            for m in range(8):
                for n in range(2):
                    acc = ppsum.tile([P, 512], F32, tag="proj", name="qp")
                    for sub in range(8):
                        nc.tensor.matmul(
                            acc,
                            lhsT=wq_bf[:, sub, m * P : (m + 1) * P],
                            rhs=xq_bf[:, sub, n * 512 : (n + 1) * 512],
                            start=(sub == 0),
                            stop=(sub == 7),
                        )
                    nc.vector.tensor_copy(
                        out=qt_sb[:, m, n * 512 : (n + 1) * 512], in_=acc
                    )

            # ---- duplicated kv prefix ------------------------------------
            if b_dup:
                xd_bf = xstream.tile([P, 8, b_dup * P], BF16, tag="xd", name="xd")
                nc.sync.dma_start(
                    out=xd_bf, in_=xd_ext[:].rearrange("(sub p) s -> p sub s", p=P)
                )
                dup_dst = lambda blk: kv_dup[blk]
                kt_proj(dup_dst, wk_bf, xd_bf, 0, b_dup * P)
                v_proj(dup_dst, wv1_bf, xd_bf, 0, b_dup * P)

        # ---- attention ----------------------------------------------------
        with (
            tc.tile_pool(name="asingles", bufs=1) as asingles,
            tc.tile_pool(name="vpool", bufs=6) as vpool,
            tc.tile_pool(name="mpool", bufs=3) as mpool,
            tc.tile_pool(name="epool", bufs=2) as epool,
            tc.tile_pool(name="gpool", bufs=2) as gpool,
            tc.tile_pool(name="ltpsum", bufs=2, space="PSUM") as ltpsum,
            tc.tile_pool(name="hpsum", bufs=1, space="PSUM") as hpsum,
            tc.tile_pool(name="spsum", bufs=1, space="PSUM") as spsum,
        ):
            wv2_bf = wload(nc, asingles, wv2_ext, "wv2")

            def visit_srcs(s, j):
                if j < b_dup:
                    base = kv_dup[j]
                else:
                    t = j // 2
                    s_own = t // 8
                    gath = kv_gathA if s_own < 2 else kv_gathB
                    base = gath[t % 8, :, s_own % 2, j % 2]
                return base[0], base[1].rearrange("p m c -> p (m c)")

            def load_visit(s, j):
                kt_src, v_src = visit_srcs(s, j)
                kt_t = vpool.tile([P, 8, P], BF16, tag="kt", name="kt_t")
                nc.sync.dma_start(out=kt_t, in_=kt_src)
                v_t = vpool.tile([P, 1024], BF16, tag="v", name="v_t")
                nc.sync.dma_start(out=v_t, in_=v_src)
                return kt_t, v_t

            def logits(s, j, kt_t):
                lt = ltpsum.tile([P, 256], F32, tag="lt", name="lt")
                for sub in range(8):
                    nc.tensor.matmul(
                        lt,
                        lhsT=kt_t[:, sub, :],
                        rhs=qt_sb[:, sub, s * 256 : (s + 1) * 256],
                        start=(sub == 0),
                        stop=(sub == 7),
                    )
                return lt

            def pv(s, j, lt, v_t, h, sums, jmax):
                pt = vpool.tile([P, 256], BF16, tag="pt", name="pt")
                nc.scalar.activation(out=pt, in_=lt, func=AF.Exp, scale=0.03125)
                if j >= 16 * s:
                    m_t = mpool.tile([P, 256], BF16, tag="m", name="m_t")
                    nc.sync.dma_start(out=m_t, in_=mask_ext[16 * s + (j - 16 * s)])
                    nc.vector.tensor_mul(out=pt, in0=pt, in1=m_t)
                for qc in range(2):
                    lhsT = pt[:, qc * P : (qc + 1) * P]
                    for dh in range(2):
                        nc.tensor.matmul(
                            h[qc][:, dh, :],
                            lhsT=lhsT,
                            rhs=v_t[:, dh * 512 : (dh + 1) * 512],
                            start=(j == 0),
                            stop=(j == jmax),
                        )
                    nc.tensor.matmul(
                        sums[:, qc : qc + 1],
                        lhsT=lhsT,
                        rhs=ones_sb,
                        start=False,
                        stop=(j == jmax),
                        skip_group_check=True,
                    )

            for s in range(NSLOTS):
                nv = 16 * (s + 1)
                jmax = nv - 1
                h = [
                    hpsum.tile([P, 2, 512], F32, tag=f"hq{qc}", name=f"h{qc}_{s}")
                    for qc in range(2)
                ]
                sums = spsum.tile([P, 2], F32, tag="sums", name="sums")
                nc.tensor.matmul(
                    sums,
                    lhsT=zcol_sb,
                    rhs=zrow_sb[:, :2],
                    start=True,
                    stop=False,
                    skip_group_check=True,
                )
                # software pipeline: logits of j+1 are emitted before pv of j
                kt_t, v_t = load_visit(s, 0)
                lt_prev = logits(s, 0, kt_t)
                v_prev = v_t
                for j in range(1, nv):
                    kt_t, v_t = load_visit(s, j)
                    lt = logits(s, j, kt_t)
                    pv(s, j - 1, lt_prev, v_prev, h, sums, jmax)
                    lt_prev, v_prev = lt, v_t
                pv(s, jmax, lt_prev, v_prev, h, sums, jmax)

                # ---- epilogue ----------------------------------------
                g_bf = []
                for qc in range(2):
                    recip = epool.tile([P, 1], F32, tag="recip", name="recip")
                    nc.vector.reciprocal(out=recip, in_=sums[:, qc : qc + 1])
                    g32 = epool.tile([P, 2, 512], F32, tag="g32", name="g32")
                    nc.vector.tensor_scalar_mul(
                        out=g32, in0=h[qc], scalar1=recip
                    )
                    g = gpool.tile([P, 1024], BF16, tag=f"g{qc}", name=f"g{qc}")
                    nc.scalar.activation(
                        out=g, in_=g32.rearrange("p a b -> p (a b)"), func=AF.Silu
                    )
                    g_bf.append(g)
                # transpose G -> gt [d-part, m, 256]
                gt_sb = epool.tile([P, 8, 256], BF16, tag="gt", name="gt")
                for m in range(8):
                    for qc in range(2):
                        tp = ltpsum.tile([P, 256], BF16, tag="lt", name="tp")
                        nc.tensor.transpose(
                            tp[:, :P],
                            g_bf[qc][:, m * P : (m + 1) * P],
                            ident_sb,
                        )
                        nc.vector.tensor_copy(
                            out=gt_sb[:, m, qc * P : (qc + 1) * P], in_=tp[:, :P]
                        )
                # output projection: O[q, d] via lhsT = gt chunks
                for qc in range(2):
                    op = hpsum.tile(
                        [P, 2, 512], F32, tag=f"hq{qc}", name=f"o{qc}_{s}"
                    )
                    for m in range(8):
                        for dh in range(2):
                            nc.tensor.matmul(
                                op[:, dh, :],
                                lhsT=gt_sb[:, m, qc * P : (qc + 1) * P],
                                rhs=wv2_bf[:, m, dh * 512 : (dh + 1) * 512],
                                start=(m == 0),
                                stop=(m == 7),
                            )
                    oo = epool.tile([P, 2, 512], F32, tag="oo", name="oo")
                    nc.vector.tensor_copy(out=oo, in_=op)
                    nc.sync.dma_start(
                        out=o_ext[s, qc], in_=oo.rearrange("p a b -> p (a b)")
                    )

        singles_ctx.__exit__(None, None, None)

    nc.finalize()
    return nc


_NC_CACHE = {}


def get_nc(b_dup=B_DUP):
    if b_dup not in _NC_CACHE:
        _NC_CACHE[b_dup] = build_kernel(b_dup)
    return _NC_CACHE[b_dup]


def build_masks():
    """Masks for the last 16 visits of each slot, selected per core by
    k = 2c + 16s - j: k>=1 all-visible, k==0 upper-left triangle, k==-1
    shifted triangle, k<=-2 fully masked (padded visit)."""
    p = np.arange(P)[:, None]
    u = np.arange(256)[None, :]
    m_ones = np.ones((P, 256), np.float32)
    m0 = (p <= u).astype(np.float32)
    m1 = (p <= u - P).astype(np.float32)
    m_zero = np.zeros((P, 256), np.float32)
    canon = np.stack([m_zero, m1, m0, m_ones]).astype(ml_dtypes.bfloat16)

    out = []
    for c in range(NCORES):
        sel = []
        for s in range(NSLOTS):
            for j in range(16 * s, 16 * (s + 1)):
                k = 2 * c + 16 * s - j
                sel.append(min(max(k, -2), 1) + 2)
        out.append(canon[np.array(sel, np.int64)])
    return out  # list of [64, 128, 256] bf16


def build_in_maps(x, wq, wk, wv1, wv2, b_dup=B_DUP):
    bf = ml_dtypes.bfloat16
    xT = np.ascontiguousarray(np.asarray(x, np.float32).T).astype(bf)
    masks = build_masks()
    xd = np.ascontiguousarray(xT[:, : b_dup * P])
    w = {
        "wq": np.asarray(wq, np.float32).astype(bf),
        "wk": np.asarray(wk, np.float32).astype(bf),
        "wv1": np.asarray(wv1, np.float32).astype(bf),
        "wv2": np.asarray(wv2, np.float32).astype(bf),
    }
    in_maps = []
    for c in range(NCORES):
        xq_c = np.concatenate(
            [xT[:, 256 * (c + 8 * s) : 256 * (c + 8 * s) + 256] for s in range(NSLOTS)],
            axis=1,
        )
        in_maps.append(
            {"xq": np.ascontiguousarray(xq_c), "xd": xd, "masks": masks[c], **w}
        )
    return in_maps


def assemble_out(results):
    out = np.empty((SEQ, D), np.float32)
    for c in range(NCORES):
        o = results[c]["o"]  # [4, 2, 128, 1024]
        for s in range(NSLOTS):
            r0 = 256 * (c + 8 * s)
            out[r0 : r0 + P, :] = o[s, 0]
            out[r0 + P : r0 + 256, :] = o[s, 1]
    return out


def kernel(x, wq, wk, wv1, wv2):
    in_maps = build_in_maps(x, wq, wk, wv1, wv2)
    nc = get_nc()
    res = run_bass_kernel_spmd(nc, in_maps, list(range(NCORES)))
    return assemble_out(res.results)
